# Initial kernel scaffold
#
"""Trainium2 Bass kernel for nn_MultiHeadAttention (B=2, T=S=2048, E=1024, H=16, D=64).

Sharding: 8 cores = 2 batches x 4 head-groups (4 heads each). Each core computes
its batch's Q/K/V projections for its 4 heads, per-head attention (scores,
softmax, attn @ V), writes its 4 heads' attention weights, and a partial output
projection (row-parallel over head features). Host sums the 4 partials per batch
and adds o_b.

Device-side layout choices (host pre-transposes so the kernel never transposes):
  - x^T [E, T] per input so the E-contraction sits on SBUF partitions.
  - Weights pre-transposed to [E, 256] (q_w pre-scaled by D^-0.5).
  - Scores are computed twice, in [t, s] layout (softmax + attn_weights output)
    and [s, t] layout (exp'd to bf16, feeds attn @ V on the PE directly).
  - Row sums come free from the Exp activation's accum_out; attn@V output is
    normalized via a broadcast reciprocal row.
"""

import numpy as np

import concourse.bacc as bacc
import concourse.bass as bass
import concourse.tile as tile
from concourse import mybir
from concourse.bass_utils import run_bass_kernel_spmd

F32 = mybir.dt.float32
F32R = mybir.dt.float32r
BF16 = mybir.dt.bfloat16
FT = mybir.ActivationFunctionType

B, T, S, E, H, D = 2, 2048, 2048, 1024, 16, 64
HL = 4          # heads per core
N_CORES = 8
KCH = E // 128  # 8 contraction chunks for projections


def _r(ap):
    return ap.bitcast(F32R)


def emit_body(nc, tc, aps, reps=1):
    (xqT, xkT, xvT, wqT, wkT, wvT, qb, kb, vb, owT, attnW, outp) = aps

    wpool = tc.tile_pool(name="weights", bufs=1)
    xpool = tc.tile_pool(name="xT", bufs=2)
    spool = tc.tile_pool(name="state", bufs=1)
    ps = tc.tile_pool(name="ps", bufs=2, space="PSUM")       # [128,1024] = 2 banks/slot
    psT = tc.tile_pool(name="psT", bufs=1, space="PSUM")     # 2 banks
    psAV = tc.tile_pool(name="psAV", bufs=1, space="PSUM")   # 2 banks
    epool = tc.tile_pool(name="exp", bufs=3)
    etpool = tc.tile_pool(name="expT", bufs=2)
    apool = tc.tile_pool(name="attn", bufs=2)
    rbpool = tc.tile_pool(name="rb", bufs=1)
    opool = tc.tile_pool(name="out", bufs=2)

    rrow_dram = nc.dram_tensor("rrow_scratch", [HL, T], F32).ap()

    for _rep in range(reps):
        # ---- weights ----
        wq = wpool.tile([128, KCH, 256], F32, tag="wq")
        nc.sync.dma_start(out=wq, in_=wqT.rearrange("(k p) m -> p k m", p=128))
        wk = wpool.tile([128, KCH, 256], F32, tag="wk")
        nc.sync.dma_start(out=wk, in_=wkT.rearrange("(k p) m -> p k m", p=128))
        wv = wpool.tile([128, KCH, 256], F32, tag="wv")
        nc.sync.dma_start(out=wv, in_=wvT.rearrange("(k p) m -> p k m", p=128))
        ow = wpool.tile([128, 2, E], F32, tag="ow")
        nc.sync.dma_start(out=ow, in_=owT.rearrange("(c p) n -> p c n", p=128))
        qb_sb = wpool.tile([128, 2], F32, tag="qb")
        nc.sync.dma_start(out=qb_sb, in_=qb.rearrange("(m p) -> p m", p=128))
        kb_sb = wpool.tile([128, 2], F32, tag="kb")
        nc.sync.dma_start(out=kb_sb, in_=kb.rearrange("(m p) -> p m", p=128))
        vb_row = wpool.tile([1, 256], F32, tag="vbr")
        nc.sync.dma_start(out=vb_row, in_=vb.rearrange("(a n) -> a n", a=1))
        ones_row = wpool.tile([1, 128], F32, tag="ones")
        nc.vector.memset(ones_row, 1.0)

        # ---- state ----
        QT = spool.tile([128, 2, T], F32, tag="QT")   # q^T, 2 m-chunks (2 heads each)
        KT = spool.tile([128, 2, S], F32, tag="KT")
        Vb = spool.tile([128, S // 128, 256], BF16, tag="Vb")  # v natural, bf16
        av_sb = spool.tile([128, 2, T], F32, tag="av")          # (attn@V)^T unnormalized
        rs = spool.tile([128, HL, T // 128], F32, tag="rs")     # row sums (t on partitions)
        rs2 = spool.tile([128, HL, T // 128, 2], F32, tag="rs2")
        rc = spool.tile([128, HL, T // 128], F32, tag="rc")     # 1/rowsum
        rrow = spool.tile([HL, T], F32, tag="rrow")             # 1/rowsum, row layout

        # ---- phase 1: projections ----
        for wt, bias_t, dst, xsrc in ((wq, qb_sb, QT, xqT), (wk, kb_sb, KT, xkT)):
            xh = []
            for kh in range(4):
                xt = xpool.tile([128, 2, T], F32, tag="xT")
                nc.sync.dma_start(
                    out=xt,
                    in_=xsrc.rearrange("(k p) t -> p k t", p=128)[:, 2 * kh : 2 * kh + 2, :],
                )
                xh.append(xt)
            for mc in range(2):
                for nt in range(4):
                    p = ps.tile([128, 512], F32, tag="ps")
                    for k in range(KCH):
                        nc.tensor.matmul(
                            p,
                            lhsT=_r(wt[:, k, mc * 128 : (mc + 1) * 128]),
                            rhs=_r(xh[k // 2][:, k % 2, nt * 512 : (nt + 1) * 512]),
                            start=(k == 0),
                            stop=(k == KCH - 1),
                        )
                    nc.vector.tensor_scalar_add(
                        out=dst[:, mc, nt * 512 : (nt + 1) * 512],
                        in0=p,
                        scalar1=bias_t[:, mc : mc + 1],
                    )
        # V projection (natural [s, d] layout), bias via rank-1 matmul
        xh = []
        for kh in range(4):
            xt = xpool.tile([128, 2, T], F32, tag="xT")
            nc.sync.dma_start(
                out=xt,
                in_=xvT.rearrange("(k p) t -> p k t", p=128)[:, 2 * kh : 2 * kh + 2, :],
            )
            xh.append(xt)
        for sc in range(S // 128):
            p = ps.tile([128, 256], F32, tag="ps")
            for k in range(KCH):
                nc.tensor.matmul(
                    p,
                    lhsT=_r(xh[k // 2][:, k % 2, sc * 128 : (sc + 1) * 128]),
                    rhs=_r(wv[:, k, :]),
                    start=(k == 0),
                    stop=False,
                )
            nc.tensor.matmul(p, lhsT=_r(ones_row), rhs=_r(vb_row), start=False, stop=True)
            nc.vector.tensor_copy(out=Vb[:, sc, :], in_=p)

        # ---- phase 2: per-head attention ----
        def b_iter(h, tc_i):
            mc, po = h // 2, (h % 2) * 64
            exs = []
            for sh in range(2):
                pb = ps.tile([128, 1024], F32, tag="ps")
                for ss in range(2):
                    nc.tensor.matmul(
                        pb[:, ss * 512 : (ss + 1) * 512],
                        lhsT=_r(QT[po : po + 64, mc, tc_i * 128 : (tc_i + 1) * 128]),
                        rhs=_r(KT[po : po + 64, mc, (sh * 2 + ss) * 512 : (sh * 2 + ss + 1) * 512]),
                        start=True,
                        stop=True,
                    )
                ex = epool.tile([128, 1024], F32, tag="exp")
                nc.scalar.activation(
                    out=ex, in_=pb, func=FT.Exp,
                    accum_out=rs2[:, h, tc_i, sh : sh + 1],
                )
                exs.append(ex)
            nc.vector.tensor_add(
                out=rs[:, h, tc_i : tc_i + 1],
                in0=rs2[:, h, tc_i, 0:1],
                in1=rs2[:, h, tc_i, 1:2],
            )
            nc.vector.reciprocal(out=rc[:, h, tc_i : tc_i + 1], in_=rs[:, h, tc_i : tc_i + 1])
            for sh, ex in enumerate(exs):
                at = apool.tile([128, 1024], F32, tag="attn")
                nc.vector.tensor_scalar_mul(out=at, in0=ex, scalar1=rc[:, h, tc_i : tc_i + 1])
                nc.sync.dma_start(
                    out=attnW[h, tc_i * 128 : (tc_i + 1) * 128, sh * 1024 : (sh + 1) * 1024],
                    in_=at,
                )

        def a_iter(h, th, sc, avp):
            mc, po = h // 2, (h % 2) * 64
            pt = psT.tile([128, 1024], F32, tag="psT")
            for tt in range(2):
                nc.tensor.matmul(
                    pt[:, tt * 512 : (tt + 1) * 512],
                    lhsT=_r(KT[po : po + 64, mc, sc * 128 : (sc + 1) * 128]),
                    rhs=_r(QT[po : po + 64, mc, th * 1024 + tt * 512 : th * 1024 + (tt + 1) * 512]),
                    start=True,
                    stop=True,
                )
            ext = etpool.tile([128, 1024], BF16, tag="expT")
            nc.scalar.activation(out=ext, in_=pt, func=FT.Exp)
            for tq in range(2):
                nc.tensor.matmul(
                    avp[po : po + 64, tq * 512 : (tq + 1) * 512],
                    lhsT=Vb[:, sc, h * 64 : (h + 1) * 64],
                    rhs=ext[:, tq * 512 : (tq + 1) * 512],
                    start=(sc == 0),
                    stop=(sc == S // 128 - 1),
                )

        for h in range(HL):
            mc, po = h // 2, (h % 2) * 64
            for th in range(2):
                avp = psAV.tile([128, 1024], F32, tag="av")
                for sc in range(S // 128):
                    a_iter(h, th, sc, avp)
                    if sc % 2 == 1:
                        b_iter(h, th * 8 + sc // 2)
                nc.vector.tensor_copy(
                    out=av_sb[po : po + 64, mc, th * 1024 : (th + 1) * 1024],
                    in_=avp[po : po + 64, :],
                )

        # ---- normalize (attn@V)^T by 1/rowsum (broadcast along partitions) ----
        # scatter rc [128, HL, 16] (t-major on partitions) -> row layout [HL, T] via DRAM
        nc.sync.dma_start(out=rrow_dram.rearrange("h (c p) -> h c p", p=128),
                          in_=rc.rearrange("p h c -> h c p"))
        nc.sync.dma_start(out=rrow, in_=rrow_dram)
        for h in range(HL):
            mc, po = h // 2, (h % 2) * 64
            rb = rbpool.tile([128, T], F32, tag="rb")
            bcast_src = bass.AP(
                tensor=rrow_dram.tensor,
                offset=rrow_dram[h : h + 1, :].offset,
                ap=[[0, 128]] + list(rrow_dram[h, :].ap),
            )
            nc.sync.dma_start(out=rb, in_=bcast_src)
            nc.vector.tensor_mul(
                out=av_sb[po : po + 64, mc, :],
                in0=av_sb[po : po + 64, mc, :],
                in1=rb[po : po + 64, :],
            )

        # ---- phase 3: output projection (partial; host adds o_b and reduces) ----
        for tc_i in range(T // 128):
            p = ps.tile([128, 1024], F32, tag="ps")
            for mc in range(2):
                for ng in range(2):
                    nc.tensor.matmul(
                        p[:, ng * 512 : (ng + 1) * 512],
                        lhsT=_r(av_sb[:, mc, tc_i * 128 : (tc_i + 1) * 128]),
                        rhs=_r(ow[:, mc, ng * 512 : (ng + 1) * 512]),
                        start=(mc == 0),
                        stop=(mc == 1),
                    )
            o = opool.tile([128, 1024], F32, tag="o")
            nc.vector.tensor_copy(out=o, in_=p)
            nc.sync.dma_start(out=outp[tc_i * 128 : (tc_i + 1) * 128, :], in_=o)


def build_program(reps=1):
    nc = bacc.Bacc("TRN2", target_bir_lowering=False, debug=False, num_devices=N_CORES)
    f = F32
    xqT = nc.dram_tensor("xqT", [E, T], f, kind="ExternalInput").ap()
    xkT = nc.dram_tensor("xkT", [E, S], f, kind="ExternalInput").ap()
    xvT = nc.dram_tensor("xvT", [E, S], f, kind="ExternalInput").ap()
    wqT = nc.dram_tensor("wqT", [E, 256], f, kind="ExternalInput").ap()
    wkT = nc.dram_tensor("wkT", [E, 256], f, kind="ExternalInput").ap()
    wvT = nc.dram_tensor("wvT", [E, 256], f, kind="ExternalInput").ap()
    qb = nc.dram_tensor("qb", [256], f, kind="ExternalInput").ap()
    kb = nc.dram_tensor("kb", [256], f, kind="ExternalInput").ap()
    vb = nc.dram_tensor("vb", [256], f, kind="ExternalInput").ap()
    owT = nc.dram_tensor("owT", [256, E], f, kind="ExternalInput").ap()
    attnW = nc.dram_tensor("attnW", [HL, T, S], f, kind="ExternalOutput").ap()
    outp = nc.dram_tensor("outp", [T, E], f, kind="ExternalOutput").ap()
    aps = (xqT, xkT, xvT, wqT, wkT, wvT, qb, kb, vb, owT, attnW, outp)
    with tile.TileContext(nc) as tc:
        emit_body(nc, tc, aps, reps=reps)
    nc.compile()
    return nc


_PROGRAMS = {}


def get_program(reps=1):
    if reps not in _PROGRAMS:
        _PROGRAMS[reps] = build_program(reps)
    return _PROGRAMS[reps]


def make_in_maps(inputs):
    q = np.asarray(inputs["query"], np.float32)
    k = np.asarray(inputs.get("key_", inputs.get("key")), np.float32)
    v = np.asarray(inputs["value"], np.float32)
    q_w = np.asarray(inputs["q_w"], np.float32)
    q_b = np.asarray(inputs["q_b"], np.float32)
    k_w = np.asarray(inputs["k_w"], np.float32)
    k_b = np.asarray(inputs["k_b"], np.float32)
    v_w = np.asarray(inputs["v_w"], np.float32)
    v_b = np.asarray(inputs["v_b"], np.float32)
    o_w = np.asarray(inputs["o_w"], np.float32)
    scale = np.float32(D ** -0.5)

    xT = {}
    for b in range(B):
        xT[("q", b)] = np.ascontiguousarray(q[b].T)
        xT[("k", b)] = np.ascontiguousarray(k[b].T)
        xT[("v", b)] = np.ascontiguousarray(v[b].T)

    in_maps = []
    for c in range(N_CORES):
        b, hg = c // 4, c % 4
        fs = slice(hg * 256, (hg + 1) * 256)
        in_maps.append({
            "xqT": xT[("q", b)],
            "xkT": xT[("k", b)],
            "xvT": xT[("v", b)],
            "wqT": np.ascontiguousarray((q_w[fs] * scale).T),
            "wkT": np.ascontiguousarray(k_w[fs].T),
            "wvT": np.ascontiguousarray(v_w[fs].T),
            "qb": np.ascontiguousarray(q_b[fs] * scale),
            "kb": np.ascontiguousarray(k_b[fs]),
            "vb": np.ascontiguousarray(v_b[fs]),
            "owT": np.ascontiguousarray(o_w[:, fs].T),
        })
    return in_maps


def assemble(results, o_b):
    output = np.zeros((B, T, E), np.float32)
    attn = np.empty((B, H, T, S), np.float32)
    for c in range(N_CORES):
        b, hg = c // 4, c % 4
        attn[b, hg * 4 : (hg + 1) * 4] = results[c]["attnW"]
        output[b] += results[c]["outp"]
    output += np.asarray(o_b, np.float32)
    return output, attn


def kernel(**inputs):
    nc = get_program(reps=1)
    in_maps = make_in_maps(inputs)
    res = run_bass_kernel_spmd(nc, in_maps, core_ids=list(range(N_CORES)))
    return assemble(res.results, inputs["o_b"])


# revision 11
# speedup vs baseline: 9.4397x; 9.4397x over previous
"""Trainium2 Bass kernel for nn_MultiHeadAttention (B=2, T=S=2048, E=1024, H=16, D=64).

Sharding: 8 cores = 2 batches x 4 head-groups (4 heads each). Each core computes
its batch's Q/K/V projections for its 4 heads, per-head attention (scores,
softmax, attn @ V), writes its 4 heads' attention weights, and a partial output
projection (row-parallel over head features). Host sums the 4 partials per batch
and adds o_b.

Device-side layout choices (host pre-transposes so the kernel never transposes):
  - x^T [E, T] per input so the E-contraction sits on SBUF partitions.
  - Weights pre-transposed to [E, 256] (q_w pre-scaled by D^-0.5).
  - Scores are computed twice, in [t, s] layout (softmax + attn_weights output)
    and [s, t] layout (exp'd to bf16, feeds attn @ V on the PE directly).
  - Row sums come free from the Exp activation's accum_out; attn@V output is
    normalized via a broadcast reciprocal row.
"""

import numpy as np

import concourse.bacc as bacc
import concourse.bass as bass
import concourse.tile as tile
from concourse import mybir
from concourse.bass_utils import run_bass_kernel_spmd

F32 = mybir.dt.float32
F32R = mybir.dt.float32r
BF16 = mybir.dt.bfloat16
FT = mybir.ActivationFunctionType

B, T, S, E, H, D = 2, 2048, 2048, 1024, 16, 64
HL = 4          # heads per core
N_CORES = 8
KCH = E // 128  # 8 contraction chunks for projections


def emit_body(nc, tc, aps, reps=1):
    from contextlib import ExitStack

    (xqT, xkT, xvT, wqT, wkT, wvT, qb, kb, vb, owT, attnW, outp) = aps

    ctx = ExitStack()
    wpool = ctx.enter_context(tc.tile_pool(name="weights", bufs=1))
    xpool = ctx.enter_context(tc.tile_pool(name="xT", bufs=4))
    spool = ctx.enter_context(tc.tile_pool(name="state", bufs=1))
    ps = ctx.enter_context(tc.tile_pool(name="ps", bufs=2, space="PSUM"))    # [128,1024] = 2 banks/slot
    psT = ctx.enter_context(tc.tile_pool(name="psT", bufs=1, space="PSUM"))  # 2 banks
    psAV = ctx.enter_context(tc.tile_pool(name="psAV", bufs=1, space="PSUM"))  # 2 banks
    epool = ctx.enter_context(tc.tile_pool(name="exp", bufs=3))
    etpool = ctx.enter_context(tc.tile_pool(name="expT", bufs=2))
    apool = ctx.enter_context(tc.tile_pool(name="attn", bufs=2))
    rbpool = ctx.enter_context(tc.tile_pool(name="rb", bufs=1))
    opool = ctx.enter_context(tc.tile_pool(name="out", bufs=2))

    rrow_dram = nc.dram_tensor("rrow_scratch", [HL, T], F32).ap()

    for _rep in range(reps):
        # ---- weights ----
        wq = wpool.tile([128, KCH, 256], F32R, tag="wq")
        nc.sync.dma_start(out=wq, in_=wqT.rearrange("(k p) m -> p k m", p=128))
        wk = wpool.tile([128, KCH, 256], F32R, tag="wk")
        nc.sync.dma_start(out=wk, in_=wkT.rearrange("(k p) m -> p k m", p=128))
        wv = wpool.tile([128, KCH, 256], F32R, tag="wv")
        nc.sync.dma_start(out=wv, in_=wvT.rearrange("(k p) m -> p k m", p=128))
        ow = wpool.tile([128, 2, E], F32R, tag="ow")
        nc.sync.dma_start(out=ow, in_=owT.rearrange("(c p) n -> p c n", p=128))
        qb_sb = wpool.tile([128, 2], F32, tag="qb")
        nc.sync.dma_start(out=qb_sb, in_=qb.rearrange("(m p) -> p m", p=128))
        kb_sb = wpool.tile([128, 2], F32, tag="kb")
        nc.sync.dma_start(out=kb_sb, in_=kb.rearrange("(m p) -> p m", p=128))
        vb_row = wpool.tile([1, 256], F32R, tag="vbr")
        nc.sync.dma_start(out=vb_row, in_=vb.rearrange("(a n) -> a n", a=1))
        ones_f32 = wpool.tile([1, 128], F32, tag="ones32")
        nc.vector.memset(ones_f32, 1.0)
        ones_row = wpool.tile([1, 128], F32R, tag="ones")
        nc.vector.tensor_copy(out=ones_row, in_=ones_f32)

        # ---- state ----
        QT = spool.tile([128, 2, T], F32R, tag="QT")   # q^T, 2 m-chunks (2 heads each)
        KT = spool.tile([128, 2, S], F32R, tag="KT")
        Vb = spool.tile([128, S // 128, 256], BF16, tag="Vb")  # v natural, bf16
        av_sb = spool.tile([128, 2, T], F32R, tag="av")          # (attn@V)^T unnormalized
        rs = spool.tile([128, HL, T // 128], F32, tag="rs")     # row sums (t on partitions)
        rs2 = spool.tile([128, HL, T // 128, 2], F32, tag="rs2")
        rc = spool.tile([128, HL, T // 128], F32, tag="rc")     # 1/rowsum

        # ---- phase 1: projections ----
        for wt, bias_t, dst, xsrc in ((wq, qb_sb, QT, xqT), (wk, kb_sb, KT, xkT)):
            xh = []
            for kh in range(4):
                xt = xpool.tile([128, 2, T], F32R, tag="xT")
                nc.sync.dma_start(
                    out=xt,
                    in_=xsrc.rearrange("(k p) t -> p k t", p=128)[:, 2 * kh : 2 * kh + 2, :],
                )
                xh.append(xt)
            for mc in range(2):
                for nt in range(4):
                    p = ps.tile([128, 512], F32, tag="ps")
                    for k in range(KCH):
                        nc.tensor.matmul(
                            p,
                            lhsT=(wt[:, k, mc * 128 : (mc + 1) * 128]),
                            rhs=(xh[k // 2][:, k % 2, nt * 512 : (nt + 1) * 512]),
                            start=(k == 0),
                            stop=(k == KCH - 1),
                        )
                    nc.vector.tensor_scalar_add(
                        out=dst[:, mc, nt * 512 : (nt + 1) * 512],
                        in0=p,
                        scalar1=bias_t[:, mc : mc + 1],
                    )
        # V projection (natural [s, d] layout), bias via rank-1 matmul
        xh = []
        for kh in range(4):
            xt = xpool.tile([128, 2, T], F32R, tag="xT")
            nc.sync.dma_start(
                out=xt,
                in_=xvT.rearrange("(k p) t -> p k t", p=128)[:, 2 * kh : 2 * kh + 2, :],
            )
            xh.append(xt)
        for sc in range(S // 128):
            p = ps.tile([128, 256], F32, tag="ps")
            for k in range(KCH):
                nc.tensor.matmul(
                    p,
                    lhsT=(xh[k // 2][:, k % 2, sc * 128 : (sc + 1) * 128]),
                    rhs=(wv[:, k, :]),
                    start=(k == 0),
                    stop=False,
                )
            nc.tensor.matmul(p, lhsT=(ones_row), rhs=(vb_row), start=False, stop=True)
            nc.vector.tensor_copy(out=Vb[:, sc, :], in_=p)

        # ---- phase 2: per-head attention ----
        def b_iter(h, tc_i):
            mc, po = h // 2, (h % 2) * 64
            exs = []
            for sh in range(2):
                pb = ps.tile([128, 1024], F32, tag="ps")
                for ss in range(2):
                    nc.tensor.matmul(
                        pb[:, ss * 512 : (ss + 1) * 512],
                        lhsT=(QT[po : po + 64, mc, tc_i * 128 : (tc_i + 1) * 128]),
                        rhs=(KT[po : po + 64, mc, (sh * 2 + ss) * 512 : (sh * 2 + ss + 1) * 512]),
                        start=True,
                        stop=True,
                    )
                ex = epool.tile([128, 1024], F32, tag="exp")
                nc.scalar.activation(
                    out=ex, in_=pb, func=FT.Exp,
                    accum_out=rs2[:, h, tc_i, sh : sh + 1],
                )
                exs.append(ex)
            nc.vector.tensor_add(
                out=rs[:, h, tc_i : tc_i + 1],
                in0=rs2[:, h, tc_i, 0:1],
                in1=rs2[:, h, tc_i, 1:2],
            )
            nc.vector.reciprocal(out=rc[:, h, tc_i : tc_i + 1], in_=rs[:, h, tc_i : tc_i + 1])
            for sh, ex in enumerate(exs):
                at = apool.tile([128, 1024], F32, tag="attn")
                nc.vector.tensor_scalar_mul(out=at, in0=ex, scalar1=rc[:, h, tc_i : tc_i + 1])
                nc.sync.dma_start(
                    out=attnW[h, tc_i * 128 : (tc_i + 1) * 128, sh * 1024 : (sh + 1) * 1024],
                    in_=at,
                )

        def a_iter(h, th, sc, avp):
            mc, po = h // 2, (h % 2) * 64
            pt = psT.tile([128, 1024], F32, tag="psT")
            for tt in range(2):
                nc.tensor.matmul(
                    pt[:, tt * 512 : (tt + 1) * 512],
                    lhsT=(KT[po : po + 64, mc, sc * 128 : (sc + 1) * 128]),
                    rhs=(QT[po : po + 64, mc, th * 1024 + tt * 512 : th * 1024 + (tt + 1) * 512]),
                    start=True,
                    stop=True,
                )
            ext = etpool.tile([128, 1024], BF16, tag="expT")
            nc.scalar.activation(out=ext, in_=pt, func=FT.Exp)
            for tq in range(2):
                nc.tensor.matmul(
                    avp[po : po + 64, tq * 512 : (tq + 1) * 512],
                    lhsT=Vb[:, sc, h * 64 : (h + 1) * 64],
                    rhs=ext[:, tq * 512 : (tq + 1) * 512],
                    start=(sc == 0),
                    stop=(sc == S // 128 - 1),
                )

        for h in range(HL):
            mc, po = h // 2, (h % 2) * 64
            for th in range(2):
                avp = psAV.tile([128, 1024], F32, tag="av")
                for sc in range(S // 128):
                    a_iter(h, th, sc, avp)
                    if sc % 2 == 1:
                        b_iter(h, th * 8 + sc // 2)
                nc.vector.tensor_copy(
                    out=av_sb[po : po + 64, mc, th * 1024 : (th + 1) * 1024],
                    in_=avp[po : po + 64, :],
                )

        # ---- normalize (attn@V)^T by 1/rowsum (broadcast along partitions) ----
        # scatter rc [128, HL, 16] (t-major on partitions) -> row layout [HL, T] via DRAM
        for h in range(HL):
            nc.sync.dma_start(out=rrow_dram[h].rearrange("(c p) -> p c", p=128),
                              in_=rc[:, h, :])
        for h in range(HL):
            mc, po = h // 2, (h % 2) * 64
            rb = rbpool.tile([128, T], F32, tag="rb")
            bcast_src = bass.AP(
                tensor=rrow_dram.tensor,
                offset=rrow_dram[h : h + 1, :].offset,
                ap=[[0, 128]] + list(rrow_dram[h, :].ap),
            )
            nc.sync.dma_start(out=rb, in_=bcast_src)
            nc.vector.tensor_mul(
                out=av_sb[po : po + 64, mc, :],
                in0=av_sb[po : po + 64, mc, :],
                in1=rb[po : po + 64, :],
            )

        # ---- phase 3: output projection (partial; host adds o_b and reduces) ----
        for tc_i in range(T // 128):
            p = ps.tile([128, 1024], F32, tag="ps")
            for mc in range(2):
                for ng in range(2):
                    nc.tensor.matmul(
                        p[:, ng * 512 : (ng + 1) * 512],
                        lhsT=(av_sb[:, mc, tc_i * 128 : (tc_i + 1) * 128]),
                        rhs=(ow[:, mc, ng * 512 : (ng + 1) * 512]),
                        start=(mc == 0),
                        stop=(mc == 1),
                    )
            o = opool.tile([128, 1024], F32, tag="o")
            nc.vector.tensor_copy(out=o, in_=p)
            nc.sync.dma_start(out=outp[tc_i * 128 : (tc_i + 1) * 128, :], in_=o)

    ctx.close()


def build_program(reps=1):
    nc = bacc.Bacc("TRN2", target_bir_lowering=False, debug=False, num_devices=N_CORES)
    f = F32
    xqT = nc.dram_tensor("xqT", [E, T], F32R, kind="ExternalInput").ap()
    xkT = nc.dram_tensor("xkT", [E, S], F32R, kind="ExternalInput").ap()
    xvT = nc.dram_tensor("xvT", [E, S], F32R, kind="ExternalInput").ap()
    wqT = nc.dram_tensor("wqT", [E, 256], F32R, kind="ExternalInput").ap()
    wkT = nc.dram_tensor("wkT", [E, 256], F32R, kind="ExternalInput").ap()
    wvT = nc.dram_tensor("wvT", [E, 256], F32R, kind="ExternalInput").ap()
    qb = nc.dram_tensor("qb", [256], f, kind="ExternalInput").ap()
    kb = nc.dram_tensor("kb", [256], f, kind="ExternalInput").ap()
    vb = nc.dram_tensor("vb", [256], F32R, kind="ExternalInput").ap()
    owT = nc.dram_tensor("owT", [256, E], F32R, kind="ExternalInput").ap()
    attnW = nc.dram_tensor("attnW", [HL, T, S], f, kind="ExternalOutput").ap()
    outp = nc.dram_tensor("outp", [T, E], f, kind="ExternalOutput").ap()
    aps = (xqT, xkT, xvT, wqT, wkT, wvT, qb, kb, vb, owT, attnW, outp)
    with tile.TileContext(nc) as tc:
        emit_body(nc, tc, aps, reps=reps)
    nc.compile()
    return nc


_PROGRAMS = {}


def get_program(reps=1):
    if reps not in _PROGRAMS:
        _PROGRAMS[reps] = build_program(reps)
    return _PROGRAMS[reps]


def make_in_maps(inputs):
    q = np.asarray(inputs["query"], np.float32)
    k = np.asarray(inputs.get("key_", inputs.get("key")), np.float32)
    v = np.asarray(inputs["value"], np.float32)
    q_w = np.asarray(inputs["q_w"], np.float32)
    q_b = np.asarray(inputs["q_b"], np.float32)
    k_w = np.asarray(inputs["k_w"], np.float32)
    k_b = np.asarray(inputs["k_b"], np.float32)
    v_w = np.asarray(inputs["v_w"], np.float32)
    v_b = np.asarray(inputs["v_b"], np.float32)
    o_w = np.asarray(inputs["o_w"], np.float32)
    scale = np.float32(D ** -0.5)

    xT = {}
    for b in range(B):
        xT[("q", b)] = np.ascontiguousarray(q[b].T)
        xT[("k", b)] = np.ascontiguousarray(k[b].T)
        xT[("v", b)] = np.ascontiguousarray(v[b].T)

    in_maps = []
    for c in range(N_CORES):
        b, hg = c // 4, c % 4
        fs = slice(hg * 256, (hg + 1) * 256)
        in_maps.append({
            "xqT": xT[("q", b)],
            "xkT": xT[("k", b)],
            "xvT": xT[("v", b)],
            "wqT": np.ascontiguousarray((q_w[fs] * scale).T),
            "wkT": np.ascontiguousarray(k_w[fs].T),
            "wvT": np.ascontiguousarray(v_w[fs].T),
            "qb": np.ascontiguousarray(q_b[fs] * scale),
            "kb": np.ascontiguousarray(k_b[fs]),
            "vb": np.ascontiguousarray(v_b[fs]),
            "owT": np.ascontiguousarray(o_w[:, fs].T),
        })
    return in_maps


def assemble(results, o_b):
    output = np.zeros((B, T, E), np.float32)
    attn = np.empty((B, H, T, S), np.float32)
    for c in range(N_CORES):
        b, hg = c // 4, c % 4
        attn[b, hg * 4 : (hg + 1) * 4] = results[c]["attnW"]
        output[b] += results[c]["outp"]
    output += np.asarray(o_b, np.float32)
    return output, attn


def kernel(**inputs):
    nc = get_program(reps=1)
    in_maps = make_in_maps(inputs)
    res = run_bass_kernel_spmd(nc, in_maps, core_ids=list(range(N_CORES)))
    return assemble(res.results, inputs["o_b"])
